# revision 23
# baseline (speedup 1.0000x reference)
"""Distributed Bass kernel for nn_CausalGraphVAE on 8 TRN2 NeuronCores.

Node dimension sharded 8 ways (256 nodes/core). Major algebraic
restructurings vs the reference:
  - H=0 makes the TGCN R-gate dead code; out = (1-Z)*Ht.
  - Wm = h3^T h3 factorizes through gl_proj_end: with h2a=[h2|1],
    Wge_aug=[Wge;bge], Wm = Wge_aug^T (h2a^T h2a) Wge_aug -> a [257,257]
    Gram all-reduce instead of a 51.5 GFLOP matmul + 16.8MB reduce.
  - AhT @ (x @ W) = (AhT @ x) @ W everywhere (aggregation first).
  - Wadj symmetric -> deg = local rowsum; AhT columns = transposed local
    rows (PE transpose).
  - Host-folded weights: conf_dec into tgcn1, cz.w @ lz.w[:N] for
    tgcn_dec, qkv packing, 1/sqrt(dh) into Wq.
  - A output is a pure function of A_score -> computed on host.
Compute dtype: bf16 matmuls (f32 PSUM accumulate), f32 for the Wm/Wadj
chain (float32r PE path), f32 elementwise for gates/softmax/deg.
"""
import sys, os, types, ctypes, contextlib, functools

for _p in ("/opt/trn_rl_repo",):
    if _p not in sys.path and os.path.isdir(_p):
        sys.path.insert(0, _p)

import numpy as np
import ml_dtypes

# ---- antenv.axon_hooks shim (lets trace=True capture NTFF profiles) ----
def _install_hook_shim():
    if "antenv.axon_hooks" in sys.modules:
        return
    holder = {}
    m = types.ModuleType("antenv.axon_hooks")
    m.set_axon_ntff_profile_hook = lambda h: holder.__setitem__("h", h)
    m.get_axon_ntff_profile_hook = lambda: holder.get("h")
    sys.modules["antenv.axon_hooks"] = m
    so_path = "/opt/axon/libaxon_pjrt.so"
    if not os.path.exists(so_path):
        return
    try:
        lib = ctypes.CDLL(so_path)
        if not hasattr(lib, "axon_start_nrt_profile"):
            return
        lib.axon_start_nrt_profile.argtypes = [ctypes.POINTER(ctypes.c_int64),
                                               ctypes.c_size_t]
        lib.axon_start_nrt_profile.restype = ctypes.c_int64
        lib.axon_stop_nrt_profile.argtypes = [ctypes.c_char_p]
        lib.axon_stop_nrt_profile.restype = ctypes.c_int64

        @contextlib.contextmanager
        def _hook(output_dir, device_ids):
            import jax
            jax.devices()
            if device_ids:
                ids = (ctypes.c_int64 * len(device_ids))(*device_ids)
                rc = lib.axon_start_nrt_profile(ids, len(device_ids))
            else:
                rc = lib.axon_start_nrt_profile(None, 0)
            if rc != 0:
                raise RuntimeError(f"axon_start_nrt_profile rc={rc}")
            try:
                yield
            finally:
                n = lib.axon_stop_nrt_profile(str(output_dir).encode())
                print(f"profile: {n} file(s) -> {output_dir}", file=sys.stderr)

        m.set_axon_ntff_profile_hook(_hook)
    except OSError:
        pass

_install_hook_shim()

import concourse.bass as bass
import concourse.bacc as bacc
import concourse.mybir as mybir
import concourse.tile as tile
from concourse.bass_utils import run_bass_kernel_spmd
from concourse.masks import make_identity

# surface real compile errors from inside the PJRT compile callback
import traceback as _tb
import concourse.bass2jax as _b2j
_orig_ncc_hook = _b2j.neuronx_cc_hook

def _loud_ncc_hook(*a, **k):
    try:
        return _orig_ncc_hook(*a, **k)
    except BaseException:
        with open("/tmp/ncc_hook_error.txt", "w") as f:
            _tb.print_exc(file=f)
        raise

_b2j.neuronx_cc_hook = _loud_ncc_hook

M = 8
N = 2048
NL = N // M            # 256 local nodes
E = 128
H1 = 256
H2 = 256
LAT = 64
P = 3
NH = 4
DH = H1 // NH          # 64
SMALL = 1e-8
PN = P * N             # 6144

BF = mybir.dt.bfloat16
F32 = mybir.dt.float32
F32R = mybir.dt.float32r
AF = mybir.ActivationFunctionType
OP = mybir.AluOpType
RG = [list(range(M))]

NPBF = np.dtype(ml_dtypes.bfloat16)


def _bf(a):
    return np.ascontiguousarray(np.asarray(a, np.float32)).astype(NPBF)


def _f32(a):
    return np.ascontiguousarray(np.asarray(a, np.float32))


def f32r(ap):
    return ap.bitcast(F32R) if ap.dtype == F32 else ap


# --------------------------------------------------------------------------
# Host-side weight preparation (pure functions of params -> shared arrays)
# --------------------------------------------------------------------------
def host_prep(params):
    g = {}

    def W(p):
        return np.asarray(p["w"], np.float32)

    def B(p):
        return np.asarray(p["b"], np.float32)

    sc = 1.0 / np.sqrt(DH)
    mha = params["gl_mha"]
    # [Wgs | Wce]: x-side matmul computes h1 and conf-enc in one pass
    g["Wgse"] = _bf(np.concatenate([W(params["gl_proj_start"]),
                                    W(params["conf_enc"])], axis=1))   # [2048,320]
    g["bgse"] = _f32(np.concatenate([B(params["gl_proj_start"]),
                                     B(params["conf_enc"])]))[:, None]  # [320,1]
    g["Wqkv"] = _bf(np.concatenate([W(mha["q"]) * sc, W(mha["k"]),
                                    W(mha["v"])], axis=1))             # [256,768]
    g["bqkv"] = _f32(np.concatenate([B(mha["q"]) * sc, B(mha["k"]),
                                     B(mha["v"])]))[:, None]           # [768,1]
    g["Wo"] = _bf(W(mha["o"]))                                         # [256,256]
    g["borow"] = _bf(B(mha["o"]))[None, :]                            # [1,256]
    # block-diag head-broadcast matrix: BDB[hd', hd] = (head(hd')==head(hd))
    hd = np.arange(H1)
    g["BDB"] = _bf((hd[:, None] // DH == hd[None, :] // DH))           # [256,256]
    # augmented gl_proj_end
    wge, bge = W(params["gl_proj_end"]), B(params["gl_proj_end"])
    g["Wge_aug"] = _f32(np.concatenate([wge, bge[None, :],
                                        np.zeros((1, N), np.float32)], 0))  # [258,2048]

    g["Went"] = _bf(W(params["ent"]))                                  # [128,256]
    g["bent"] = _f32(B(params["ent"]))[:, None]
    g["Wtim"] = _bf(W(params["tim"]))
    g["btim"] = _f32(B(params["tim"]))[:, None]

    t1 = params["tgcn1"]
    wcd, bcd = W(params["conf_dec"]), B(params["conf_dec"])
    enc_w, ee_w, te_w, bc_w, wl_w, bl_w = [], [], [], [], [], []
    for cz, lz in (("cz", "lz"), ("ch", "lh")):
        Wc, bc = W(t1[cz]), B(t1[cz])
        Wl, bl = W(t1[lz]), B(t1[lz])
        enc_w.append(wcd @ Wc[:N])
        ee_w.append(Wc[N:N + H1])
        te_w.append(Wc[N + H1:])
        bc_w.append(bcd @ Wc[:N] + bc)
        wl_w.append(Wl[:H1])
        bl_w.append(bl)
    g["Wt1enc"] = _bf(np.concatenate(enc_w, 1))                        # [64,512]
    g["Wt1ee"] = _bf(np.concatenate(ee_w, 1))                          # [256,512]
    g["Wt1te"] = _bf(np.concatenate(te_w, 1))                          # [256,512]
    g["bt1row"] = _bf(np.concatenate(bc_w))[None, :]                  # [1,512]
    g["Wl1z"] = _bf(wl_w[0]); g["Wl1h"] = _bf(wl_w[1])                 # [256,256]
    g["blzrow"] = _bf(bl_w[0])[None, :]; g["blhrow"] = _bf(bl_w[1])[None, :]
    att = np.asarray(params["att"], np.float32)
    e = np.exp(att - att.max()); probs = e / e.sum()
    g["probsB"] = _f32(np.tile(probs[None, :], (128, 1)))              # [128,3]

    aW, aV, aB = [], [], []
    for nm in ("arma1", "arma2"):
        a = params[nm]
        for t in range(3):
            aW.append(np.asarray(a["W"][t], np.float32))
            aV.append(np.asarray(a["V"][t], np.float32))
            aB.append(np.asarray(a["b"][t], np.float32))
    g["armaW"] = _bf(np.stack(aW))                                     # [6,256,256]
    g["armaV"] = _bf(np.stack(aV))                                     # [6,256,256]
    g["armaB"] = _bf(np.stack(aB))[:, None, :]                        # [6,1,256]

    g["Wmulv"] = _bf(np.concatenate([W(params["mu"]), W(params["logvar"])], 1))
    g["bmulv"] = _bf(np.concatenate([B(params["mu"]), B(params["logvar"])]))[None, :]
    # arma1 L3 produces h3 in feature-major form; bias is per-partition there
    g["b3col_1"] = _f32(aB[2])[:, None]                                # [256,1]
    g["b3row_2"] = _f32(aB[5])[None, :]                                # [1,256]
    g["Wdf"] = _bf(W(params["dec_fc"]))                                # [64,256]
    g["bdfrow"] = _f32(B(params["dec_fc"]))[None, :]                   # [1,256]

    td = params["tgcn_dec"]
    F_, bF = [], []
    for cz, lz in (("cz", "lz"), ("ch", "lh")):
        Wc, bc = W(td[cz]), B(td[cz])
        Wl, bl = W(td[lz]), B(td[lz])
        F_.append(Wc @ Wl[:N])
        bF.append(bc @ Wl[:N] + bl)
    g["Fzh"] = _bf(np.stack(F_))                                       # [2,256,2048]
    g["btd"] = _bf(np.stack(bF))[:, None, :]                          # [2,1,2048]
    return g


def host_eps():
    """jax.random.normal(key(42), (N, LAT)) on CPU — matches the reference."""
    import jax
    cpus = jax.devices("cpu")
    with jax.default_device(cpus[0]):
        return np.asarray(jax.random.normal(jax.random.key(42), (N, LAT),
                                            np.float32))


def host_A(A_score):
    a = np.asarray(A_score, np.float32) * (1.0 - np.eye(N, dtype=np.float32))
    y = np.clip(a, 0.0, 1.0)
    return (y + (a == 0.0).astype(np.float32) * SMALL).astype(np.float32)


# --------------------------------------------------------------------------
# Kernel graph
# --------------------------------------------------------------------------
_IN_SPECS = [
    # per-core
    ("xt", [P, N, NL], BF), ("embT", [P, 2 * E, NL], BF),
    ("eps_l", [NL, LAT], F32), ("Wge_loc", [258, NL], F32R),
    ("diagmask", [NL, N], F32), ("eyeT", [N, NL], BF),
    # shared
    ("Wgse", [N, 320], BF), ("bgse", [320, 1], F32),
    ("Wqkv", [H1, 3 * H1], BF), ("bqkv", [3 * H1, 1], F32),
    ("Wo", [H1, H1], BF), ("borow", [1, H1], BF), ("BDB", [H1, H1], BF),
    ("Wge_aug", [258, N], F32R),
    ("Went", [E, H1], BF), ("bent", [H1, 1], F32),
    ("Wtim", [E, H1], BF), ("btim", [H1, 1], F32),
    ("Wt1enc", [LAT, 2 * H1], BF), ("Wt1ee", [H1, 2 * H1], BF),
    ("Wt1te", [H1, 2 * H1], BF), ("bt1row", [1, 2 * H1], BF),
    ("Wl1z", [H1, H1], BF), ("Wl1h", [H1, H1], BF),
    ("blzrow", [1, H1], BF), ("blhrow", [1, H1], BF),
    ("probsB", [128, P], F32),
    ("armaW", [6, H1, H1], BF), ("armaV", [6, H1, H1], BF),
    ("armaB", [6, 1, H1], BF),
    ("Wmulv", [H1, 2 * LAT], BF), ("bmulv", [1, 2 * LAT], BF),
    ("b3col_1", [H1, 1], F32),
    ("Wdf", [LAT, H2], BF), ("bdfrow", [1, H2], F32),
    ("Fzh", [2, H2, N], BF), ("btd", [2, 1, N], BF),
]


def build_nc():
    nc = bacc.Bacc("TRN2", target_bir_lowering=False, debug=False, num_devices=M)
    t = {}
    for name, shape, dt in _IN_SPECS:
        t[name] = nc.dram_tensor(name, shape, dt, kind="ExternalInput").ap()
    outs = [("wadj_o", [NL, N], F32), ("recon_o", [NL, N], F32),
            ("mulv_o", [NL, 2 * LAT], F32)]
    if os.environ.get("KERNEL_DEBUG"):
        outs += [("dbg_bf", [NL, 1536], BF), ("dbg_z", [NL, LAT], F32),
                 ("dbg3", [NL, 1536], BF)]
    for name, shape, dt in outs:
        t[name] = nc.dram_tensor(name, shape, dt, kind="ExternalOutput").ap()
    t["_debug"] = bool(os.environ.get("KERNEL_DEBUG"))

    with tile.TileContext(nc) as tc:
        _build_graph(nc, tc, t)
    nc.compile()
    return nc


def _build_graph(nc, tc, t):
    ctx = contextlib.ExitStack()
    with ctx:
        _build_graph_inner(nc, tc, t, ctx)


def _build_graph_inner(nc, tc, t, ctx):
    sync, vec, act, ten, gp = nc.sync, nc.vector, nc.scalar, nc.tensor, nc.gpsimd

    # ------- pools (PSUM: ps 4 + ps2 2 + psT 2 = 8 banks exactly) -------
    wmain = ctx.enter_context(tc.tile_pool(name="wmain", bufs=1))
    smain = ctx.enter_context(tc.tile_pool(name="smain", bufs=1))
    tmp = ctx.enter_context(tc.tile_pool(name="tmp", bufs=2))
    gpool = ctx.enter_context(tc.tile_pool(name="gpool", bufs=4))
    ps = ctx.enter_context(tc.tile_pool(name="ps", bufs=4, space="PSUM"))
    ps2 = ctx.enter_context(tc.tile_pool(name="ps2", bufs=2, space="PSUM"))
    psT = ctx.enter_context(tc.tile_pool(name="psT", bufs=2, space="PSUM"))
    dram = ctx.enter_context(tc.tile_pool(name="dram", bufs=1, space="DRAM"))
    drsh = ctx.enter_context(tc.tile_pool(name="drsh", bufs=1, space="DRAM"))

    def wt(pool, src_ap, shape, dt, name):
        w = pool.tile(shape, dt, name=name)
        sync.dma_start(w[:], src_ap)
        return w

    def pst(shape, tag="ps"):
        return ps.tile(shape, F32, name="pt", tag=tag)

    def ps2t(shape):
        return ps2.tile(shape, F32, name="pt2", tag="ps2")

    TT = dict(start=True, stop=True)

    # ------- constants -------
    ident = wmain.tile([128, 128], F32, name="ident")
    make_identity(nc, ident[:])
    ident_bf = wmain.tile([128, 128], BF, name="ident_bf")
    act.copy(ident_bf[:], ident[:])
    ones1 = wmain.tile([1, 128], F32, name="ones1")
    vec.memset(ones1[:], 1.0)
    ones_bf = wmain.tile([1, 128], BF, name="ones_bf")
    vec.memset(ones_bf[:], 1.0)
    onz = wmain.tile([128, 2], F32, name="onz")
    vec.memset(onz[:, 0:1], 1.0)
    vec.memset(onz[:, 1:2], 0.0)

    # ------- collective bounce buffers -------
    gaug_in = dram.tile([258, 258], F32, name="gaug_in")
    gaug_out = drsh.tile([258, 258], F32, addr_space="Shared", name="gaug_out")
    deg_in = dram.tile([NL, 1], F32, name="deg_in")
    deg_out = drsh.tile([N, 1], F32, addr_space="Shared", name="deg_out")
    xw_in = dram.tile([NL, P * 512], BF, name="xw_in")
    xw_out = drsh.tile([N, P * 512], BF, addr_space="Shared", name="xw_out")
    ag_in = [dram.tile([NL, H1], BF, name=f"ag_in{i}") for i in range(7)]
    ag_out = [drsh.tile([N, H1], BF, addr_space="Shared", name=f"ag_out{i}")
              for i in range(7)]

    MSL3 = [(0, 128), (128, 128), (256, 64)]     # fo tiles of Wgse
    KSL3 = [(0, 128), (128, 128), (256, 2)]      # K tiles of the padded 258-dim

    # =====================================================================
    # PHASE 1+2+G: x matmuls, MHA, Gram partial, AllReduce
    # =====================================================================
    with tc.tile_pool(name="wx", bufs=1) as wx, \
         tc.tile_pool(name="spx", bufs=1) as spx, \
         tc.tile_pool(name="xstream", bufs=18) as xstream:
        h2a = [spx.tile([128, 258], F32R, name=f"h2a_{i}") for i in range(6)]
        wg = [wt(wx, t["Wgse"][k * 128:(k + 1) * 128, :], [128, 320], BF, f"wg{k}")
              for k in range(16)]
        bg = [wt(wx, t["bgse"][mo:mo + sz, :], [sz, 1], F32, f"bg{i}")
              for i, (mo, sz) in enumerate(MSL3)]
        h1_fm = [[spx.tile([128, NL], BF, name=f"h1_{p}_{i}") for i in range(2)]
                 for p in range(P)]
        enc_fm = [smain.tile([64, NL], BF, name=f"enc_{p}") for p in range(P)]
        for p in range(P):
            xts = []
            for k in range(16):
                xk = xstream.tile([128, NL], BF, name="xk", tag="xk")
                sync.dma_start(xk[:], t["xt"][p, k * 128:(k + 1) * 128, :])
                xts.append(xk)
            for mi, (mo, sz) in enumerate(MSL3):
                pt = pst([sz, NL])
                for k in range(16):
                    ten.matmul(pt[:], wg[k][:, mo:mo + sz], xts[k][:],
                               start=(k == 0), stop=(k == 15))
                dst = h1_fm[p][mi] if mi < 2 else enc_fm[p]
                act.activation(dst[:], pt[:], AF.Identity, bias=bg[mi][:])

        wqkv = [wt(wx, t["Wqkv"][k * 128:(k + 1) * 128, :], [128, 768], BF,
                   f"wqkv{k}") for k in range(2)]
        bqk = [wt(wx, t["bqkv"][m * 128:(m + 1) * 128, :], [128, 1], F32,
                  f"bqk{m}") for m in range(6)]
        bdb = [wt(wx, t["BDB"][k * 128:(k + 1) * 128, :], [128, 256], BF,
                  f"bdb{k}") for k in range(2)]
        wo = [wt(wx, t["Wo"][k * 128:(k + 1) * 128, :], [128, 256], BF,
                 f"wo{k}") for k in range(2)]
        borow = wt(wx, t["borow"][:], [1, 256], BF, "borow_t")

        qkv = [[spx.tile([128, NL], BF, name=f"qkv_{p}_{m}") for m in range(6)]
               for p in range(P)]
        for p in range(P):
            for m in range(6):
                pt = pst([128, NL])
                for k in range(2):
                    ten.matmul(pt[:], wqkv[k][:, m * 128:(m + 1) * 128],
                               h1_fm[p][k][:], start=(k == 0), stop=(k == 1))
                act.activation(qkv[p][m][:], pt[:], AF.Identity, bias=bqk[m][:])

        for l in range(P):
            e_sb = []
            for m in range(P):
                qk = [tmp.tile([128, NL], BF, name="qk", tag="qk")
                      for _ in range(2)]
                for i in range(2):
                    vec.tensor_mul(qk[i][:], qkv[l][i][:], qkv[m][2 + i][:])
                em = []
                for mi in range(2):
                    # BDB is block-diagonal on 128-tiles: only k == mi contributes
                    pt = pst([128, NL])
                    ten.matmul(pt[:], bdb[mi][:, mi * 128:(mi + 1) * 128],
                               qk[mi][:], **TT)
                    e_i = tmp.tile([128, NL], F32, name="e_i", tag=f"em{m}_{mi}", bufs=1)
                    act.activation(e_i[:], pt[:], AF.Exp)
                    em.append(e_i)
                e_sb.append(em)
            o_l = [tmp.tile([128, NL], BF, name="o_l", tag=f"ol{i}")
                   for i in range(2)]
            for i in range(2):
                s = tmp.tile([128, NL], F32, name="s", tag="s_sm")
                vec.tensor_add(s[:], e_sb[0][i][:], e_sb[1][i][:])
                vec.tensor_add(s[:], s[:], e_sb[2][i][:])
                r = tmp.tile([128, NL], F32, name="r", tag="r_sm")
                vec.reciprocal(r[:], s[:])
                for m in range(P):
                    a_m = tmp.tile([128, NL], BF, name="a_m", tag="a_sm")
                    vec.tensor_mul(a_m[:], e_sb[m][i][:], r[:])
                    av = tmp.tile([128, NL], BF, name="av", tag="av_sm")
                    vec.tensor_mul(av[:], a_m[:], qkv[m][4 + i][:])
                    if m == 0:
                        vec.tensor_copy(o_l[i][:], av[:])
                    else:
                        vec.tensor_add(o_l[i][:], o_l[i][:], av[:])
            for jt in range(2):
                pt = pst([128, 256])
                for k in range(2):
                    ten.matmul(pt[:], o_l[k][:, jt * 128:(jt + 1) * 128],
                               wo[k][:], start=(k == 0), stop=False)
                ten.matmul(pt[:], ones_bf[:], borow[:],
                           start=False, stop=True)
                dst = h2a[l * 2 + jt]
                act.copy(dst[:, 0:256], pt[:])
                act.copy(dst[:, 256:258], onz[:])

        # G_aug partial = sum_k h2a[k]^T h2a[k]  -> DRAM -> AllReduce
        for mi, (mo, sz) in enumerate(KSL3):
            pt = ps2t([sz, 258])
            for k in range(6):
                ten.matmul(pt[:], f32r(h2a[k][:, mo:mo + sz]), f32r(h2a[k][:]),
                           start=(k == 0), stop=(k == 5))
            gch = tmp.tile([sz, 258], F32, name="gch", tag="gch")
            act.copy(gch[:], pt[:])
            sync.dma_start(gaug_in[mo:mo + sz, :], gch[:])
    gp.collective_compute("AllReduce", OP.add, replica_groups=RG,
                          ins=[gaug_in[:].opt()], outs=[gaug_out[:].opt()])

    # =====================================================================
    # PHASE 3: Wm rows -> Wadj, deg, S_A / S_G
    # =====================================================================
    S_A = [smain.tile([128, NL], BF, name=f"S_A{gt}") for gt in range(16)]
    S_G = [smain.tile([128, NL], BF, name=f"S_G{gt}") for gt in range(16)]
    dinvA_g = smain.tile([128, 16], F32, name="dinvA_g")
    dinvG_g = smain.tile([128, 16], F32, name="dinvG_g")
    dinvB_A = smain.tile([128, NL], F32, name="dinvB_A")
    dinvB_G = smain.tile([128, NL], F32, name="dinvB_G")

    with tc.tile_pool(name="wwm", bufs=2) as wwm, \
         tc.tile_pool(name="spwm", bufs=1) as spwm:
        ga = [spwm.tile([sz, 258], F32R, name=f"ga{i}")
              for i, (mo, sz) in enumerate(KSL3)]
        for i, (mo, sz) in enumerate(KSL3):
            gaf = tmp.tile([sz, 258], F32, name="gaf", tag="gaf", bufs=1)
            sync.dma_start(gaf[:], gaug_out[mo:mo + sz, :])
            act.copy(ga[i][:], gaf[:])
        wgl = [wt(spwm, t["Wge_loc"][mo:mo + sz, :], [sz, NL], F32R, f"wgl{i}")
               for i, (mo, sz) in enumerate(KSL3)]

        wadj_nat = [spwm.tile([128, N], F32, name=f"wadj{jt}") for jt in range(2)]
        deg_l = [spwm.tile([128, 1], F32, name=f"deg_l{jt}") for jt in range(2)]
        # chunked over N: M1a chunk = G_aug @ Wge_aug[:, nsl], then both
        # Wm row-tiles for that chunk. Wge_aug streamed per chunk.
        for nch in range(4):
            nsl = slice(nch * 512, (nch + 1) * 512)
            wgac = [wwm.tile([sz, 512], F32R, name="wgac", tag=f"wgac{i}")
                    for i, (mo, sz) in enumerate(KSL3)]
            for i, (mo, sz) in enumerate(KSL3):
                sync.dma_start(wgac[i][:], t["Wge_aug"][mo:mo + sz, nsl])
            m1c = [wwm.tile([sz, 512], F32R, name="m1c", tag=f"m1c{i}")
                   for i, (mo, sz) in enumerate(KSL3)]
            for mi, (mo, sz) in enumerate(KSL3):
                pt = ps2t([sz, 512])
                for k in range(3):
                    ten.matmul(pt[:], ga[k][:, mo:mo + sz],
                               wgac[k][:], start=(k == 0), stop=(k == 2))
                act.copy(m1c[mi][:], pt[:])
            for jt in range(2):
                jsl = slice(jt * 128, (jt + 1) * 128)
                pt = ps2t([128, 512])
                for k in range(3):
                    ten.matmul(pt[:], wgl[k][:, jsl], m1c[k][:],
                               start=(k == 0), stop=(k == 2))
                act.copy(wadj_nat[jt][:, nsl], pt[:])
        for jt in range(2):
            jsl = slice(jt * 128, (jt + 1) * 128)
            dm = tmp.tile([128, N], F32, name="dm", tag="dm", bufs=1)
            sync.dma_start(dm[:], t["diagmask"][jsl, :])
            vec.tensor_mul(wadj_nat[jt][:], wadj_nat[jt][:], dm[:])
            iz = tmp.tile([128, N], F32, name="iz", tag="iz", bufs=1)
            vec.tensor_scalar(iz[:], wadj_nat[jt][:], 0.0, None, OP.is_equal)
            vec.tensor_scalar(wadj_nat[jt][:], wadj_nat[jt][:], 0.0, 1.0,
                              OP.max, OP.min)
            vec.scalar_tensor_tensor(wadj_nat[jt][:], iz[:], SMALL,
                                     wadj_nat[jt][:], OP.mult, OP.add,
                                     accum_out=deg_l[jt][:])
            sync.dma_start(t["wadj_o"][jsl, :], wadj_nat[jt][:])
            sync.dma_start(deg_in[jsl, :], deg_l[jt][:])
        gp.collective_compute("AllGather", OP.bypass, replica_groups=RG,
                              ins=[deg_in[:].opt()], outs=[deg_out[:].opt()])

        # dinv per-partition over g: [128, 16]
        deg_g = spwm.tile([128, 16], F32, name="deg_g")
        sync.dma_start(deg_g[:], deg_out[:].rearrange("(a b) 1 -> b a", b=128))
        sq = tmp.tile([128, 16], F32, name="sq", tag="sq")
        act.sqrt(sq[:], deg_g[:])
        vec.reciprocal(dinvA_g[:], sq[:])
        act.activation(sq[:], deg_g[:], AF.Sqrt, bias=1.0)
        vec.reciprocal(dinvG_g[:], sq[:])

        # local-j deg as a row: read back the contiguous deg_in DRAM buffer
        dgrow = tmp.tile([1, NL], F32, name="dgrow", tag="dgrow")
        sync.dma_start(dgrow[:], deg_in[:].rearrange("a 1 -> 1 a"))
        for bias, dst in ((0.0, dinvB_A), (1.0, dinvB_G)):
            srow = tmp.tile([1, NL], F32, name="srow", tag="srow")
            act.activation(srow[:], dgrow[:], AF.Sqrt, bias=bias)
            rrow = tmp.tile([1, NL], F32, name="rrow", tag="rrow")
            vec.reciprocal(rrow[:], srow[:])
            pb = psT.tile([128, NL], F32, name="pb", tag="psT")
            ten.matmul(pb[:], ones1[:], rrow[:], **TT)
            act.copy(dst[:], pb[:])

        # S_A / S_G tiles
        for gt in range(16):
            gsl = slice(gt * 128, (gt + 1) * 128)
            eyt = gpool.tile([128, NL], BF, name="eyt", tag="eyt")
            sync.dma_start(eyt[:], t["eyeT"][gsl, :])
            ptr = psT.tile([128, NL], F32, name="ptr", tag="psT")
            for jt in range(2):
                ten.matmul(ptr[:, jt * 128:(jt + 1) * 128],
                           wadj_nat[jt][:, gsl], ident[:], is_transpose=True,
                           start=True, stop=False, skip_group_check=True)
            tmpA = tmp.tile([128, NL], F32, name="tmpA", tag="tmpA")
            act.activation(tmpA[:], ptr[:], AF.Copy, scale=dinvA_g[:, gt:gt + 1])
            vec.tensor_mul(S_A[gt][:], tmpA[:], dinvB_A[:])
            tmpG = tmp.tile([128, NL], F32, name="tmpG", tag="tmpG")
            act.activation(tmpG[:], ptr[:], AF.Copy, scale=dinvG_g[:, gt:gt + 1])
            # + I*dinvG[g] on DVE: transpose-mode PSUM writes don't accumulate
            # with a subsequent normal matmul, so the eye must be added here.
            vec.scalar_tensor_tensor(tmpG[:], eyt[:], dinvG_g[:, gt:gt + 1],
                                     tmpG[:], OP.mult, OP.add)
            vec.tensor_mul(S_G[gt][:], tmpG[:], dinvB_G[:])
            if t["_debug"] and gt in (0, 8):
                sync.dma_start(
                    t["dbg3"][0:128, (gt // 8) * 256:(gt // 8) * 256 + 256],
                    S_G[gt][:])

    # =====================================================================
    # PHASE 4: embeddings + xw natural [j, 1536] -> AllGather
    # =====================================================================
    went = wt(wmain, t["Went"][:], [128, 256], BF, "went_t")
    wtim = wt(wmain, t["Wtim"][:], [128, 256], BF, "wtim_t")
    bent = [wt(wmain, t["bent"][m * 128:(m + 1) * 128, :], [128, 1], F32,
               f"bent{m}") for m in range(2)]
    btim = [wt(wmain, t["btim"][m * 128:(m + 1) * 128, :], [128, 1], F32,
               f"btim{m}") for m in range(2)]
    w1enc = wt(wmain, t["Wt1enc"][:], [64, 512], BF, "w1enc")
    w1ee = [wt(wmain, t["Wt1ee"][k * 128:(k + 1) * 128, :], [128, 512], BF,
               f"w1ee{k}") for k in range(2)]
    w1te = [wt(wmain, t["Wt1te"][k * 128:(k + 1) * 128, :], [128, 512], BF,
               f"w1te{k}") for k in range(2)]
    bt1 = wt(wmain, t["bt1row"][:], [1, 512], BF, "bt1_t")

    xw_sb = [smain.tile([128, P * 512], BF, name=f"xw{jt}") for jt in range(2)]
    for p in range(P):
        emb_ent = gpool.tile([128, NL], BF, name="emb_ent", tag="emb_e")
        sync.dma_start(emb_ent[:], t["embT"][p, 0:128, :])
        emb_tim = gpool.tile([128, NL], BF, name="emb_tim", tag="emb_t")
        sync.dma_start(emb_tim[:], t["embT"][p, 128:256, :])
        ee = [tmp.tile([128, NL], BF, name="ee", tag=f"ee{m}") for m in range(2)]
        te = [tmp.tile([128, NL], BF, name="te", tag=f"te{m}") for m in range(2)]
        for m in range(2):
            msl = slice(m * 128, (m + 1) * 128)
            pt = pst([128, NL])
            ten.matmul(pt[:], went[:, msl], emb_ent[:], **TT)
            act.activation(ee[m][:], pt[:], AF.Relu, bias=bent[m][:])
            pt2 = pst([128, NL])
            ten.matmul(pt2[:], wtim[:, msl], emb_tim[:], **TT)
            act.activation(te[m][:], pt2[:], AF.Relu, bias=btim[m][:])
        for jt in range(2):
            jsl = slice(jt * 128, (jt + 1) * 128)
            pt = ps2t([128, 512])
            ten.matmul(pt[:], enc_fm[p][:, jsl], w1enc[:], start=True, stop=False)
            for k in range(2):
                ten.matmul(pt[:], ee[k][:, jsl], w1ee[k][:], start=False,
                           stop=False)
                ten.matmul(pt[:], te[k][:, jsl], w1te[k][:], start=False,
                           stop=False)
            ten.matmul(pt[:], ones_bf[:], bt1[:], start=False, stop=True)
            act.copy(xw_sb[jt][:, p * 512:(p + 1) * 512], pt[:])
    for jt in range(2):
        sync.dma_start(xw_in[jt * 128:(jt + 1) * 128, :], xw_sb[jt][:])
    gp.collective_compute("AllGather", OP.bypass, replica_groups=RG,
                          ins=[xw_in[:].opt()], outs=[xw_out[:].opt()])

    # =====================================================================
    # PHASE 5: tgcn1 aggregation + gates + h
    # =====================================================================
    wl1z = [wt(wmain, t["Wl1z"][k * 128:(k + 1) * 128, :], [128, 256], BF,
               f"wl1z{k}") for k in range(2)]
    wl1h = [wt(wmain, t["Wl1h"][k * 128:(k + 1) * 128, :], [128, 256], BF,
               f"wl1h{k}") for k in range(2)]
    blz = wt(wmain, t["blzrow"][:], [1, 256], BF, "blz_t")
    blh = wt(wmain, t["blhrow"][:], [1, 256], BF, "blh_t")
    probs = wt(wmain, t["probsB"][:], [128, P], F32, "probs_t")

    h_acc = [smain.tile([128, H1], F32, name=f"hacc{jt}") for jt in range(2)]
    with tc.tile_pool(name="xgp", bufs=1) as xgp:
        xwg = []
        for k in range(16):
            xg = xgp.tile([128, P * 512], BF, name=f"xg{k}")
            sync.dma_start(xg[:], xw_out[k * 128:(k + 1) * 128, :])
            xwg.append(xg)
        for p in range(P):
            agg = [tmp.tile([128, NL], BF, name="agg", tag=f"agg{mt}")
                   for mt in range(4)]
            for mt in range(4):
                pt = pst([128, NL])
                for k in range(16):
                    ten.matmul(pt[:],
                               xwg[k][:, p * 512 + mt * 128:p * 512 + (mt + 1) * 128],
                               S_G[k][:], start=(k == 0), stop=(k == 15))
                act.copy(agg[mt][:], pt[:])
                if t["_debug"] and p == 0 and mt == 0:
                    sync.dma_start(t["dbg3"][0:128, 1024:1280], agg[mt][:])
                    sync.dma_start(t["dbg3"][128:256, 0:256], S_G[0][:])
            for jt in range(2):
                jsl = slice(jt * 128, (jt + 1) * 128)
                ptz = pst([128, H1])
                for k in range(2):
                    ten.matmul(ptz[:], agg[k][:, jsl], wl1z[k][:],
                               start=(k == 0), stop=False)
                ten.matmul(ptz[:], ones_bf[:], blz[:], start=False,
                           stop=True)
                zt = tmp.tile([128, H1], F32, name="zt", tag="zt")
                act.activation(zt[:], ptz[:], AF.Sigmoid)
                pth = pst([128, H1])
                for k in range(2):
                    ten.matmul(pth[:], agg[2 + k][:, jsl], wl1h[k][:],
                               start=(k == 0), stop=False)
                ten.matmul(pth[:], ones_bf[:], blh[:], start=False,
                           stop=True)
                ht = tmp.tile([128, H1], F32, name="ht", tag="ht")
                act.activation(ht[:], pth[:], AF.Tanh)
                vec.tensor_scalar(zt[:], zt[:], -1.0, 1.0, OP.mult, OP.add)
                vec.tensor_mul(zt[:], zt[:], ht[:])
                if p == 0:
                    vec.tensor_scalar(h_acc[jt][:], zt[:], probs[:, 0:1], None,
                                      OP.mult)
                else:
                    vec.scalar_tensor_tensor(h_acc[jt][:], zt[:],
                                             probs[:, p:p + 1], h_acc[jt][:],
                                             OP.mult, OP.add)

    h_nat = [smain.tile([128, H1], BF, name=f"hnat{jt}") for jt in range(2)]
    for jt in range(2):
        vec.tensor_relu(h_acc[jt][:], h_acc[jt][:])
        act.copy(h_nat[jt][:], h_acc[jt][:])
        sync.dma_start(ag_in[0][jt * 128:(jt + 1) * 128, :], h_nat[jt][:])
    h_fm0 = [smain.tile([128, NL], BF, name=f"hfm0_{k}") for k in range(2)]
    for k in range(2):
        ptr = psT.tile([128, NL], F32, name="ptr2", tag="psT")
        for jt in range(2):
            ten.matmul(ptr[:, jt * 128:(jt + 1) * 128],
                       h_acc[jt][:, k * 128:(k + 1) * 128], ident[:],
                       is_transpose=True, start=True, stop=(jt == 1),
                       skip_group_check=True)
        act.copy(h_fm0[k][:], ptr[:])

    # =====================================================================
    # PHASE 6/8: arma stacks
    # =====================================================================
    aw = [[wt(wmain, t["armaW"][i, k * 128:(k + 1) * 128, :], [128, 256], BF,
              f"aw{i}_{k}") for k in range(2)] for i in range(6)]
    av = [[wt(wmain, t["armaV"][i, k * 128:(k + 1) * 128, :], [128, 256], BF,
              f"av{i}_{k}") for k in range(2)] for i in range(6)]
    ab = [wt(wmain, t["armaB"][i, :, :], [1, 256], BF, f"ab{i}")
          for i in range(6)]

    def ag(idx):
        gp.collective_compute("AllGather", OP.bypass, replica_groups=RG,
                              ins=[ag_in[idx][:].opt()],
                              outs=[ag_out[idx][:].opt()])

    def arma_layer(i, x0_fm, ag_idx, fm_bias=None):
        hg = []
        for k in range(16):
            hgk = gpool.tile([128, H1], BF, name="hgk", tag="hgk")
            sync.dma_start(hgk[:], ag_out[ag_idx][k * 128:(k + 1) * 128, :])
            hg.append(hgk)
        agg = [tmp.tile([128, NL], BF, name="agf", tag=f"agf{mt}")
               for mt in range(2)]
        for mt in range(2):
            pt = pst([128, NL])
            for k in range(16):
                ten.matmul(pt[:], hg[k][:, mt * 128:(mt + 1) * 128], S_A[k][:],
                           start=(k == 0), stop=(k == 15))
            act.copy(agg[mt][:], pt[:])
        outs = []
        if fm_bias is None:
            for jt in range(2):
                jsl = slice(jt * 128, (jt + 1) * 128)
                pt = pst([128, H1])
                for k in range(2):
                    ten.matmul(pt[:], agg[k][:, jsl], aw[i][k][:],
                               start=(k == 0), stop=False)
                for k in range(2):
                    ten.matmul(pt[:], x0_fm[k][:, jsl], av[i][k][:],
                               start=False, stop=False)
                ten.matmul(pt[:], ones_bf[:], ab[i][:], start=False,
                           stop=True)
                o = tmp.tile([128, H1], BF, name="a_o", tag=f"a_o{jt}")
                act.activation(o[:], pt[:], AF.Relu)
                outs.append(o)
        else:
            for mt in range(2):
                msl = slice(mt * 128, (mt + 1) * 128)
                pt = pst([128, NL])
                for k in range(2):
                    ten.matmul(pt[:], aw[i][k][:, msl], agg[k][:],
                               start=(k == 0), stop=False)
                for k in range(2):
                    ten.matmul(pt[:], av[i][k][:, msl], x0_fm[k][:],
                               start=False, stop=(k == 1))
                o = smain.tile([128, NL], BF, name=f"a_of{i}_{mt}")
                act.activation(o[:], pt[:], AF.Relu, bias=fm_bias[mt][:])
                outs.append(o)
        return outs

    ag(0)
    h_next = arma_layer(0, h_fm0, 0)
    for jt in range(2):
        sync.dma_start(ag_in[1][jt * 128:(jt + 1) * 128, :], h_next[jt][:])
    ag(1)
    h_next = arma_layer(1, h_fm0, 1)
    for jt in range(2):
        sync.dma_start(ag_in[2][jt * 128:(jt + 1) * 128, :], h_next[jt][:])
    ag(2)
    b3c = [wt(wmain, t["b3col_1"][m * 128:(m + 1) * 128, :], [128, 1], F32,
              f"b3c{m}") for m in range(2)]
    h3_fm = arma_layer(2, h_fm0, 2, fm_bias=b3c)

    # =====================================================================
    # PHASE 7: mu/logvar/z/d0
    # =====================================================================
    wmulv = [wt(wmain, t["Wmulv"][k * 128:(k + 1) * 128, :], [128, 128], BF,
                f"wmulv{k}") for k in range(2)]
    bmulv = wt(wmain, t["bmulv"][:], [1, 128], BF, "bmulv_t")
    wdf = wt(wmain, t["Wdf"][:], [64, 256], BF, "wdf_t")
    bdf = wt(wmain, t["bdfrow"][:], [1, 256], F32, "bdf_t")

    z_nat = [smain.tile([128, LAT], F32, name=f"z_nat{jt}") for jt in range(2)]
    for jt in range(2):
        jsl = slice(jt * 128, (jt + 1) * 128)
        pt = pst([128, 128])
        for k in range(2):
            ten.matmul(pt[:], h3_fm[k][:, jsl], wmulv[k][:], start=(k == 0),
                       stop=False)
        ten.matmul(pt[:], ones_bf[:], bmulv[:], start=False, stop=True)
        ml = tmp.tile([128, 128], F32, name="ml", tag="ml")
        act.copy(ml[:], pt[:])
        sync.dma_start(t["mulv_o"][jsl, :], ml[:])
        ex = tmp.tile([128, LAT], F32, name="ex", tag="ex")
        act.activation(ex[:], ml[:, LAT:2 * LAT], AF.Exp, scale=0.5)
        ep = tmp.tile([128, LAT], F32, name="ep", tag="ep")
        sync.dma_start(ep[:], t["eps_l"][jsl, :])
        vec.tensor_mul(ex[:], ex[:], ep[:])
        vec.tensor_add(z_nat[jt][:], ml[:, 0:LAT], ex[:])
        if t["_debug"]:
            sync.dma_start(t["dbg_z"][jsl, :], z_nat[jt][:])

    z_fm = smain.tile([64, NL], BF, name="z_fm")
    ptz2 = psT.tile([64, NL], F32, name="ptz2", tag="psT")
    for jt in range(2):
        ten.matmul(ptz2[:, jt * 128:(jt + 1) * 128], z_nat[jt][:], ident[:],
                   is_transpose=True, start=True, stop=(jt == 1),
                   skip_group_check=True)
    act.copy(z_fm[:], ptz2[:])

    d0_fm = [smain.tile([128, NL], BF, name=f"d0fm{k}") for k in range(2)]
    for jt in range(2):
        pt = pst([128, H2])
        ten.matmul(pt[:], z_fm[:, jt * 128:(jt + 1) * 128], wdf[:], start=True,
                   stop=False)
        ten.matmul(pt[:], ones1[:], bdf[:], start=False, stop=True)
        d0n = tmp.tile([128, H2], BF, name="d0n", tag="d0n")
        act.copy(d0n[:], pt[:])
        sync.dma_start(ag_in[3][jt * 128:(jt + 1) * 128, :], d0n[:])
        if t["_debug"]:
            sync.dma_start(t["dbg_bf"][jt * 128:(jt + 1) * 128, 0:256], d0n[:])
    for mt in range(2):
        pt = pst([128, NL])
        ten.matmul(pt[:], wdf[:, mt * 128:(mt + 1) * 128], z_fm[:], **TT)
        bcol = tmp.tile([128, 1], F32, name="bcol", tag="bcol")
        sync.dma_start(bcol[:], t["bdfrow"][:, mt * 128:(mt + 1) * 128]
                       .rearrange("1 a -> a 1"))
        act.activation(d0_fm[mt][:], pt[:], AF.Identity, bias=bcol[:])

    ag(3)
    d_next = arma_layer(3, d0_fm, 3)
    for jt in range(2):
        sync.dma_start(ag_in[4][jt * 128:(jt + 1) * 128, :], d_next[jt][:])
        if t["_debug"]:
            sync.dma_start(t["dbg_bf"][jt * 128:(jt + 1) * 128, 256:512],
                           d_next[jt][:])
    ag(4)
    d_next = arma_layer(4, d0_fm, 4)
    for jt in range(2):
        sync.dma_start(ag_in[5][jt * 128:(jt + 1) * 128, :], d_next[jt][:])
        if t["_debug"]:
            sync.dma_start(t["dbg_bf"][jt * 128:(jt + 1) * 128, 512:768],
                           d_next[jt][:])
    ag(5)
    d_next = arma_layer(5, d0_fm, 5)
    for jt in range(2):
        sync.dma_start(ag_in[6][jt * 128:(jt + 1) * 128, :], d_next[jt][:])
        if t["_debug"]:
            sync.dma_start(t["dbg_bf"][jt * 128:(jt + 1) * 128, 768:1024],
                           d_next[jt][:])
    ag(6)

    # =====================================================================
    # PHASE 9: tgcn_dec -> recon rows
    # =====================================================================
    dg = []
    for k in range(16):
        dgk = gpool.tile([128, H1], BF, name="dgk", tag="hgk")
        sync.dma_start(dgk[:], ag_out[6][k * 128:(k + 1) * 128, :])
        if t["_debug"] and k in (0, 8):
            sync.dma_start(
                t["dbg3"][0:128, 512 + (k // 8) * 256:512 + (k // 8) * 256 + 256],
                dgk[:])
        dg.append(dgk)
    aggd = [tmp.tile([128, NL], BF, name="aggd", tag=f"agf{mt}")
            for mt in range(2)]
    for mt in range(2):
        pt = pst([128, NL])
        for k in range(16):
            ten.matmul(pt[:], dg[k][:, mt * 128:(mt + 1) * 128], S_G[k][:],
                       start=(k == 0), stop=(k == 15))
        act.copy(aggd[mt][:], pt[:])
        if t["_debug"]:
            sync.dma_start(
                t["dbg_bf"][0:128, 1024 + mt * 256:1024 + (mt + 1) * 256]
                .rearrange("a b -> a b"), aggd[mt][:])

    with tc.tile_pool(name="wdec", bufs=1) as wdec, \
         tc.tile_pool(name="spdec", bufs=1) as spdec:
        fz = [[wt(wdec, t["Fzh"][gi, k * 128:(k + 1) * 128, :], [128, N], BF,
                  f"fz{gi}_{k}") for k in range(2)] for gi in range(2)]
        btdr = [wt(wdec, t["btd"][gi, :, :], [1, N], BF, f"btd{gi}")
                for gi in range(2)]
        for jt in range(2):
            jsl = slice(jt * 128, (jt + 1) * 128)
            zh = []
            for gi, fn in ((0, AF.Sigmoid), (1, AF.Tanh)):
                dst = spdec.tile([128, N], F32, name="dst", tag=f"zh{gi}_{jt}")
                for nch in range(4):
                    nsl = slice(nch * 512, (nch + 1) * 512)
                    pt = ps2t([128, 512])
                    for k in range(2):
                        ten.matmul(pt[:], aggd[k][:, jsl], fz[gi][k][:, nsl],
                                   start=(k == 0), stop=False)
                    ten.matmul(pt[:], ones_bf[:], btdr[gi][:, nsl],
                               start=False, stop=True)
                    act.activation(dst[:, nsl], pt[:], fn)
                zh.append(dst)
            vec.tensor_scalar(zh[0][:], zh[0][:], -1.0, 1.0, OP.mult, OP.add)
            vec.tensor_mul(zh[0][:], zh[0][:], zh[1][:])
            vec.tensor_relu(zh[0][:], zh[0][:])
            sync.dma_start(t["recon_o"][jsl, :], zh[0][:])


# --------------------------------------------------------------------------
# Entry point
# --------------------------------------------------------------------------
_CACHE = {}


def _get_nc():
    if "nc" not in _CACHE:
        _CACHE["nc"] = build_nc()
    return _CACHE["nc"]


def kernel(x, entity_emb, time_emb, num_nodes, params, _want_trace=False):
    x = np.asarray(x, np.float32)
    ent = np.asarray(entity_emb, np.float32)
    tim = np.asarray(time_emb, np.float32)
    g = host_prep(params)
    eps = host_eps()
    eye = np.eye(N, dtype=np.float32)

    in_maps = []
    wge_aug_np = g["Wge_aug"]
    for c in range(M):
        rows = slice(c * NL, (c + 1) * NL)
        im = {k: v for k, v in g.items()
              if k not in ("b3row_2",)}
        im["xt"] = np.ascontiguousarray(
            x[:, rows, :].transpose(0, 2, 1)).astype(NPBF)
        im["embT"] = np.ascontiguousarray(np.concatenate(
            [ent[:, rows, :].transpose(0, 2, 1),
             tim[:, rows, :].transpose(0, 2, 1)], axis=1)).astype(NPBF)
        im["eps_l"] = _f32(eps[rows])
        im["Wge_loc"] = _f32(wge_aug_np[:, rows])
        im["diagmask"] = _f32(1.0 - eye[rows, :])
        im["eyeT"] = _bf(eye[:, rows])
        in_maps.append(im)

    nc = _get_nc()
    if _want_trace:
        import concourse.bass_utils as _bu
        _bu.upload_artifacts = lambda tmpdir: f"local://{tmpdir}"
    import tempfile
    res = run_bass_kernel_spmd(nc, in_maps, core_ids=list(range(M)),
                               trace=_want_trace,
                               **({"tmpdir": tempfile.mkdtemp(prefix="ktrace_")}
                                  if _want_trace else {}))
    if os.environ.get("KERNEL_DEBUG"):
        dbg = {c: {"bf": res.results[c]["dbg_bf"], "z": res.results[c]["dbg_z"],
                   "bf3": res.results[c].get("dbg3")}
               for c in range(M)}
        np.save("/tmp/kdbg.npy", dbg, allow_pickle=True)
    recon = np.concatenate([res.results[c]["recon_o"] for c in range(M)], 0)
    mulv = np.concatenate([res.results[c]["mulv_o"] for c in range(M)], 0)
    wadj = np.concatenate([res.results[c]["wadj_o"] for c in range(M)], 0)
    A = host_A(params["A_score"])
    out = (recon, mulv[:, :LAT], mulv[:, LAT:], wadj, A)
    if _want_trace:
        return out, res
    return out
